# revision 62
# baseline (speedup 1.0000x reference)
"""Trainium2 Bass kernel for CapsNet dynamic routing (nn_Capsule_34342558498916).

Full inputs:  u_vecs (64, 64, 1024) f32, W (1024, 32, 64, 16) f32
Full output:  (64, 16, 32) f32  == transpose(v, (0, 2, 1)) of v (B, N, D)

Sharding: capsule dim N=32 split across 8 cores (NL=4 each); routing's
softmax axis (u) is core-local so no collectives.

Design notes (final):
  - The routing loop amplifies logit errors ~40x through exp (fp16
    anywhere in the u_hat path measures 2-5e-2 final error, over the 2e-2
    gate), so inputs are shipped as PAIRS: fp16 main values plus fp8e5m2
    RESIDUALS (the residuals are 2^-11-scale, so fp8 fidelity on them
    keeps the effective input error ~4e-5).  Phase-1 runs 3 matmul
    streams per output accumulating exactly in PSUM fp32: x16*w16 (fp16)
    + dx8*w8 + x8*dw8 (fp8, with the fp8 copies of x/w converted on-chip
    by ACT/GpSimd under the DMA window).  DMA is 25.2 MB/core, 0.75x of
    fp32 bytes, at fp16/fp8 matmul speed (1 cyc/row).
  - u_hat, e and pr2 are stored as float32r (fp32 range, ~11-bit
    mantissa; measured: final error matches an 11-bit-mantissa model at
    1.3e-2).  FP32R matmuls stream 1 cyc/row for >=256-row moving APs
    (fp32 proper would be 4).
  - Logits are recomputed per iteration from the accumulated v-sum
    (b_k = u_hat . (v1+..+v_{k-1}) by linearity) directly on the PE:
    stationary = diag(v[:, d, n]) multiplies each partition's resident
    u_hat stream inside the array, so the logits path materializes NO
    product tensor at all (the old mul1 pass, ~34us/iter of DVE, is gone).
    One PSUM tile per capsule n lets bmax/mneg/exp of capsule n pipeline
    under the folds of capsule n+1.
  - Per-parity softmax with deferred renormalization: each partition
    subtracts its own rowmax before exp (no cross-partition exchange on
    the critical path); the parity merge applies alpha = exp(m_par - M)
    to tiny [64, .] tensors afterwards.  All cross-partition moves run as
    small PE matmuls (dup/pull matrices), never SBUF-to-SBUF DMA.
  - exp runs on ACT with fused per-partition bias and fused Z accumulation
    (accum_out), reading logits straight from PSUM; sqrt in squash is
    exp(0.5*ln(x)) so ACT stays on one function table (the table list
    handed to the placement pass is filtered so Exp/Ln/Copy all resolve
    to natural_log_exp_and_others; one load at startup, no reloads).
  - s~ fold: pr2 = e * u_hat products split DVE/GpSimd per 32-t chunk,
    folded by an fp32r identity matmul into an 8-slot PSUM accumulator.
"""

import sys

import numpy as np

for _p in ("/opt/trn_rl_repo", "/opt/pypackages"):
    if _p not in sys.path:
        sys.path.append(_p)

import concourse.bass as bass
from concourse import bacc
import concourse.mybir as mybir
from concourse import tile
from concourse.bass_utils import run_bass_kernel_spmd

# Problem dims (hardcoded per harness contract)
B, C, U, N, D = 64, 64, 1024, 32, 16
NCORES = 8
NL = N // NCORES          # 4 capsules per core
T = U // 2                # 512 u-pairs
DN = D * NL               # 64 = phase-1 matmul free dim (d, n4)
P = 128
EPS = 1e-8
ROUTINGS = 3
CHT = 32                  # mul2 chunk (t per chunk)
NCH = T // CHT            # 16 chunks
DVT = 21                  # t per chunk on DVE (rest on GpSimd)

dt = mybir.dt
AF = mybir.ActivationFunctionType
ALU = mybir.AluOpType

_COMPILED = {}
F32R = dt.float32r


def _pin_act_tables():
    """Filter the ACT-table decision copy so Exp/Ln/Copy resolve to the one
    table containing all three.  Returns a restore function."""
    orig = bacc.get_activation_tables
    pin = {"Exp", "Ln", "Copy"}

    def patched(arch):
        tables = orig(arch)
        out = {}
        for name, fns in tables.items():
            if name == "natural_log_exp_and_others":
                out[name] = fns
            else:
                out[name] = {f for f in fns if str(f).split(".")[-1] not in pin}
        return out

    bacc.get_activation_tables = patched
    return lambda: setattr(bacc, "get_activation_tables", orig)


def _squash_core(nc, sm, s_red_ap, zi_or_scale, s_sb):
    """s_red_ap: un-normalized s~ [B, D, NL]; s = s~ * zi or * scalar;
    returns v_sb = squash(s).  sqrt(x) = exp(0.5*ln(x))."""
    if isinstance(zi_or_scale, float):
        nc.vector.tensor_scalar_mul(s_sb[:], s_red_ap, zi_or_scale)
    else:
        zi_bc = zi_or_scale[:].unsqueeze(1).broadcast_to([B, D, NL])
        nc.vector.tensor_mul(s_sb[:], s_red_ap, zi_bc)
    ssq = sm.tile([B, D, NL], dt.float32, tag="ssq")
    nc.vector.tensor_mul(ssq[:], s_sb[:], s_sb[:])
    s2 = sm.tile([B, NL], dt.float32, tag="s2")
    nc.vector.tensor_reduce(
        s2[:], ssq[:].transpose([0, 2, 1]), axis=mybir.AxisListType.X, op=ALU.add
    )
    s2e = sm.tile([B, NL], dt.float32, tag="s2e")
    nc.vector.tensor_scalar_add(s2e[:], s2[:], EPS)
    lns = sm.tile([B, NL], dt.float32, tag="lns")
    nc.scalar.activation(lns[:], s2e[:], AF.Ln)
    rt = sm.tile([B, NL], dt.float32, tag="rt")
    nc.scalar.activation(rt[:], lns[:], AF.Exp, scale=0.5)
    den = sm.tile([B, NL], dt.float32, tag="den")
    nc.vector.tensor_scalar_add(den[:], s2e[:], 1.0)
    deni = sm.tile([B, NL], dt.float32, tag="deni")
    nc.vector.reciprocal(deni[:], den[:])
    f = sm.tile([B, NL], dt.float32, tag="f")
    nc.vector.tensor_mul(f[:], rt[:], deni[:])
    v_sb = sm.tile([B, D, NL], dt.float32, tag="v_sb")
    nc.vector.tensor_mul(v_sb[:], s_sb[:], f[:].unsqueeze(1).broadcast_to([B, D, NL]))
    return v_sb


def _build_program():
    restore = _pin_act_tables()
    try:
        return _build_program_inner()
    finally:
        restore()


def _build_program_inner():
    nc = bacc.Bacc()

    ut = nc.dram_tensor("ut", [P, T, B], dt.float16, kind="ExternalInput")
    dut = nc.dram_tensor("dut", [P, T, B], dt.float8e5, kind="ExternalInput")
    wt = nc.dram_tensor("wt", [P, T, DN], dt.float16, kind="ExternalInput")
    dwt = nc.dram_tensor("dwt", [P, T, DN], dt.float8e5, kind="ExternalInput")
    ident32_d = nc.dram_tensor("ident32", [P, P], dt.float32, kind="ExternalInput")
    dup32_d = nc.dram_tensor("dup32", [B, P], dt.float32, kind="ExternalInput")
    pull32_d = nc.dram_tensor("pull32", [P, B], dt.float32, kind="ExternalInput")
    out_d = nc.dram_tensor("out", [B, D, NL], dt.float32, kind="ExternalOutput")

    with tile.TileContext(nc) as tc:
        with (
            tc.tile_pool(name="big", bufs=1) as big,
            tc.tile_pool(name="sm", bufs=1) as sm,
        ):
            u_hat = big.tile([P, T, D, NL], F32R, tag="u_hat")
            e_sb = big.tile([P, T, NL], F32R, tag="e_sb")
            identR = sm.tile([P, P], F32R, tag="identR")
            ident32 = sm.tile([P, P], dt.float32, tag="ident32")
            dup32 = sm.tile([B, P], dt.float32, tag="dup32")
            pull32 = sm.tile([P, B], dt.float32, tag="pull32")
            mnegs = []
            for n in range(NL):
                mneg_n = sm.tile([P, 1], dt.float32, tag=f"mneg{n}")
                mnegs.append(mneg_n)
            bmax = sm.tile([P, NL], dt.float32, tag="bmax")
            z_p = sm.tile([P, NL], dt.float32, tag="z_p")
            z_f = sm.tile([B, NL], dt.float32, tag="z_f")
            zi = sm.tile([B, NL], dt.float32, tag="zi")
            s_sb = sm.tile([B, D, NL], dt.float32, tag="s_sb")
            vsum = sm.tile([B, D, NL], dt.float32, tag="vsum")
            s1q = sm.tile([P, 16, DN], dt.float32, tag="s1q")

            # ---------------- Phase 1: u_hat = einsum over c ----------------
            # 3 fp16 streams accumulate per PSUM slot: x16*w16 + dx*w16 +
            # x16*dw (the dropped dx*dw term is ~2^-22 relative).
            TCH = 16   # t per DMA chunk
            with (
                tc.tile_pool(name="wts", bufs=6) as wts,
                tc.tile_pool(name="pp", bufs=3, space="PSUM") as pp,
            ):
                for ch in range(T // TCH):
                    sl = slice(ch * TCH, (ch + 1) * TCH)
                    wt_ch = wts.tile([P, TCH, DN], dt.float16, tag="wt_ch")
                    nc.sync.dma_start(wt_ch[:], wt[:, sl, :])
                    ut_ch = wts.tile([P, TCH, B], dt.float16, tag="ut_ch")
                    nc.sync.dma_start(ut_ch[:], ut[:, sl, :])
                    dwt_ch = wts.tile([P, TCH, DN], dt.float8e5, tag="dwt_ch")
                    nc.sync.dma_start(dwt_ch[:], dwt[:, sl, :])
                    dut_ch = wts.tile([P, TCH, B], dt.float8e5, tag="dut_ch")
                    nc.sync.dma_start(dut_ch[:], dut[:, sl, :])
                    # fp8 copies of u/w for the correction streams (the
                    # corrections are 2^-11-scale, so 12.5%-accurate fp8
                    # operands keep the effective input error ~4e-5)
                    ut8_ch = wts.tile([P, TCH, B], dt.float8e5, tag="ut8_ch")
                    nc.scalar.copy(ut8_ch[:], ut_ch[:])
                    wt8_ch = wts.tile([P, TCH, DN], dt.float8e5, tag="wt8_ch")
                    nc.gpsimd.tensor_copy(wt8_ch[:], wt_ch[:])
                    if ch == 0:
                        # constants ride behind the first input chunk so the
                        # PE's first matmuls are never queued behind them
                        nc.sync.dma_start(ident32[:], ident32_d[:])
                        nc.sync.dma_start(dup32[:], dup32_d[:])
                        nc.sync.dma_start(pull32[:], pull32_d[:])
                        # fp32r identity for the s~ fold (1.0/0.0 are exact
                        # in fp32r; the DVE copy provides the required
                        # fp32r-rounded producer)
                        nc.vector.tensor_copy(identR[:], ident32[:])
                    for g in range(TCH // 8):
                        ps = pp.tile([P, 8, DN], dt.float32, tag="pp")
                        for j in range(8):
                            tl = g * 8 + j
                            for half in (slice(0, 64), slice(64, 128)):
                                nc.tensor.matmul(
                                    ps[half, j, :], ut_ch[half, tl, :],
                                    wt_ch[half, tl, :], start=True, stop=False,
                                )
                                nc.tensor.matmul(
                                    ps[half, j, :], dut_ch[half, tl, :],
                                    wt8_ch[half, tl, :], start=False, stop=False,
                                )
                                nc.tensor.matmul(
                                    ps[half, j, :], ut8_ch[half, tl, :],
                                    dwt_ch[half, tl, :], start=False, stop=True,
                                )
                        t0 = ch * TCH + g * 8
                        dst = u_hat[:, t0:t0 + 8, :, :]
                        src = ps[:].rearrange("p e (d n) -> p e d n", d=D)
                        if (ch * 4 + g) % 2 == 0:
                            nc.scalar.copy(dst, src)
                        else:
                            nc.vector.tensor_copy(dst, src)
                    if ch % 2 == 1:
                        # s1 partial for this 32-t sixteenth (DVE reduce)
                        q = ch // 2
                        nc.vector.tensor_reduce(
                            s1q[:, q, :],
                            u_hat[:, q * 32:(q + 1) * 32, :, :]
                                .bitcast(dt.float32)
                                .rearrange("p t d n -> p (d n) t"),
                            axis=mybir.AxisListType.X, op=ALU.add,
                        )

            # routing-phase SBUF pools (opened after the wts pool closes)
            ctxP = tc.tile_pool(name="prod", bufs=3)
            prodp = ctxP.__enter__()
            ctxB = tc.tile_pool(name="big2", bufs=1)
            big2 = ctxB.__enter__()
            diags = big2.tile([P, NL, D, P], F32R, tag="diags")

            # v parity broadcast via PE dup matmul: psV[m,f] = src[m%64,f];
            # the diag builds read psV straight from PSUM.
            ctxV = tc.tile_pool(name="psV", bufs=1, space="PSUM")
            psVp = ctxV.__enter__()
            psV_ref = [None]

            def _vbc_update(src_f32):
                psV = psVp.tile([P, DN], dt.float32, tag="psV")
                nc.tensor.matmul(
                    psV[:], dup32[:], src_f32.rearrange("b d n -> b (d n)"),
                    start=True, stop=True,
                )
                psV_ref[0] = psV

            # ---------------- Iteration 1: uniform c -> v1 ----------------
            s1h = sm.tile([P, DN], dt.float32, tag="s1h")
            nc.vector.tensor_reduce(
                s1h[:], s1q[:].transpose([0, 2, 1]),
                axis=mybir.AxisListType.X, op=ALU.add,
            )
            # fold parities on PE (pull upper 64 down, add)
            ctxD = tc.tile_pool(name="psD", bufs=1, space="PSUM")
            psDp = ctxD.__enter__()
            ps1 = psDp.tile([B, DN], dt.float32, tag="ps1")
            nc.tensor.matmul(ps1[:], pull32[:], s1h[:], start=True, stop=True)
            s1f = sm.tile([B, D, NL], dt.float32, tag="s1f")
            nc.vector.tensor_add(
                s1f[:].rearrange("b d n -> b (d n)"), s1h[0:64, :], ps1[:]
            )
            v_sb = _squash_core(nc, sm, s1f[:], 1.0 / U, s_sb)
            nc.vector.tensor_copy(vsum[:], v_sb[:])
            _vbc_update(vsum[:])

            # ---------------- Iterations 2..3 ----------------
            ctx2 = tc.tile_pool(name="psL", bufs=1, space="PSUM")
            psL = ctx2.__enter__()
            ctx3 = tc.tile_pool(name="psS", bufs=1, space="PSUM")
            psS = ctx3.__enter__()
            for it in range(1, ROUTINGS):
                # build diag stationaries: diags[p, n, d, m] = v[p,d,n]*I[p,m]
                # logits[p,t,n] = sum_d v[p,d,n]*u_hat[p,t,d,n] on the PE.
                # One PSUM tile per n so bmax/mneg/exp of capsule n pipeline
                # under the folds of capsule n+1 (no whole-tile WAR).  The
                # diag build of n+1 and the bmax of n-1 interleave on the DVE
                # so neither blocks the softmax tail.
                psV = psV_ref[0]
                vv = psV[:].rearrange("p (d n) -> p d n", d=D)
                lg = []
                for n in range(NL):
                    lg_n = psL.tile([P, T], dt.float32, tag=f"lg{n}")
                    lg.append(lg_n)

                def _diag_build(n):
                    nc.vector.tensor_mul(
                        diags[:, n, :, :],
                        ident32[:].unsqueeze(1).broadcast_to([P, D, P]),
                        vv[:, :, n].unsqueeze(2).broadcast_to([P, D, P]),
                    )

                def _softmax_tail(n):
                    nc.vector.tensor_reduce(
                        bmax[:, n:n + 1], lg[n][:].unsqueeze(1),
                        axis=mybir.AxisListType.X, op=ALU.max,
                    )
                    nc.vector.tensor_scalar_mul(
                        mnegs[n][:], bmax[:, n:n + 1], -1.0
                    )
                    # e = exp(b - parity rowmax) on ACT, per-parity Z fused
                    nc.scalar.activation(
                        e_sb[:, :, n], lg[n][:], AF.Exp,
                        bias=mnegs[n][:], scale=1.0,
                        accum_out=z_p[:, n:n + 1],
                    )

                _diag_build(0)
                _diag_build(1)
                for n in range(NL):
                    for d in range(D):
                        nc.tensor.matmul(
                            lg[n][:, :], diags[:, n, d, :],
                            u_hat[:, :, d, n],
                            start=(d == 0), stop=(d == D - 1),
                        )
                    if n + 2 < NL:
                        _diag_build(n + 2)
                    _softmax_tail(n)

                # s~ per parity: pr2 = e*u_hat (fp32r), identity fold on PE.
                # The parity merge (m/Z pulldowns + alpha/Z chain) is emitted
                # after the first two product chunks: it runs in the products'
                # shadow without delaying the first fold2 matmuls.
                psD = psDp.tile([B, 2 * NL + DN], dt.float32, tag="psD")
                s_ps = psS.tile([P, 8, D, NL], dt.float32, tag="s_ps")
                for c4 in range(NCH):
                    if c4 == 2:
                        nc.tensor.matmul(psD[:, 0:NL], pull32[:], bmax[:],
                                         start=True, stop=True)
                        nc.tensor.matmul(psD[:, NL:2 * NL], pull32[:], z_p[:],
                                         start=True, stop=True)
                        m_hi = psD[:, 0:NL]
                        z_hi = psD[:, NL:2 * NL]
                        # alpha_par = exp(m_par - M),  M = max(m_lo, m_hi)
                        Mfull = sm.tile([B, NL], dt.float32, tag="Mfull")
                        nc.vector.tensor_tensor(
                            Mfull[:], bmax[0:64, :], m_hi, op=ALU.max)
                        dlo = sm.tile([B, NL], dt.float32, tag="dlo")
                        nc.vector.tensor_sub(dlo[:], bmax[0:64, :], Mfull[:])
                        dhi = sm.tile([B, NL], dt.float32, tag="dhi")
                        nc.vector.tensor_sub(dhi[:], m_hi, Mfull[:])
                        alo = sm.tile([B, NL], dt.float32, tag="alo")
                        nc.scalar.activation(alo[:], dlo[:], AF.Exp)
                        ahi = sm.tile([B, NL], dt.float32, tag="ahi")
                        nc.scalar.activation(ahi[:], dhi[:], AF.Exp)
                        # Z = alo*z_lo + ahi*z_hi ; zi = 1/Z
                        zt0 = sm.tile([B, NL], dt.float32, tag="zt0")
                        nc.vector.tensor_mul(zt0[:], z_p[0:64, :], alo[:])
                        zt1 = sm.tile([B, NL], dt.float32, tag="zt1")
                        nc.vector.tensor_mul(zt1[:], z_hi, ahi[:])
                        nc.vector.tensor_add(z_f[:], zt0[:], zt1[:])
                        nc.vector.reciprocal(zi[:], z_f[:])
                    tb = c4 * CHT
                    pr2 = prodp.tile([P, CHT, D, NL], F32R, tag="pr2")
                    nc.vector.tensor_mul(
                        pr2[:, 0:DVT, :, :],
                        u_hat[:, tb:tb + DVT, :, :].bitcast(dt.float32),
                        e_sb[:, tb:tb + DVT, :].bitcast(dt.float32).unsqueeze(2)
                            .broadcast_to([P, DVT, D, NL]),
                    )
                    nc.gpsimd.tensor_mul(
                        pr2[:, DVT:CHT, :, :],
                        u_hat[:, tb + DVT:tb + CHT, :, :].bitcast(dt.float32),
                        e_sb[:, tb + DVT:tb + CHT, :].bitcast(dt.float32)
                            .unsqueeze(2)
                            .broadcast_to([P, CHT - DVT, D, NL]),
                    )
                    for g in range(CHT // 8):
                        nc.tensor.matmul(
                            s_ps[:], identR[:], pr2[:, g * 8:(g + 1) * 8, :, :],
                            start=(c4 == 0 and g == 0),
                            stop=(c4 == NCH - 1 and g == CHT // 8 - 1),
                        )
                # fold slots per parity: s_pp [128, D, NL]
                s_pp = sm.tile([P, D, NL], dt.float32, tag="s_pp")
                nc.vector.tensor_reduce(
                    s_pp[:], s_ps[:].transpose([0, 2, 3, 1]),
                    axis=mybir.AxisListType.X, op=ALU.add,
                )
                # pull upper-parity s down to rows 0:64 on the PE
                nc.tensor.matmul(
                    psD[:, 2 * NL:], pull32[:],
                    s_pp[:].rearrange("p d n -> p (d n)"),
                    start=True, stop=True,
                )
                s_hi = psD[:, 2 * NL:].rearrange("b (d n) -> b d n", d=D)
                # s_red = alo*s_lo + ahi*s_hi
                st0 = sm.tile([B, D, NL], dt.float32, tag="st0")
                nc.vector.tensor_mul(
                    st0[:], s_pp[0:64, :, :],
                    alo[:].unsqueeze(1).broadcast_to([B, D, NL]),
                )
                s_red = sm.tile([B, D, NL], dt.float32, tag="s_red")
                nc.vector.tensor_mul(
                    s_red[:], s_hi,
                    ahi[:].unsqueeze(1).broadcast_to([B, D, NL]),
                )
                nc.vector.tensor_add(s_red[:], s_red[:], st0[:])
                v_sb = _squash_core(nc, sm, s_red[:], zi, s_sb)
                if it < ROUTINGS - 1:
                    nc.vector.tensor_add(vsum[:], vsum[:], v_sb[:])
                    _vbc_update(vsum[:])

            nc.sync.dma_start(out_d[:], v_sb[:])
            ctx3.__exit__(None, None, None)
            ctx2.__exit__(None, None, None)
            ctxD.__exit__(None, None, None)
            ctxV.__exit__(None, None, None)
            ctxB.__exit__(None, None, None)
            ctxP.__exit__(None, None, None)

    nc.finalize()
    return nc


def _prep_inputs(u_vecs, W):
    """Host-side shard + relayout.  fp16 pairs (value + residual)."""
    import ml_dtypes
    f8 = ml_dtypes.float8_e5m2
    u32 = np.asarray(u_vecs, dtype=np.float32)
    utc = u32.transpose(1, 2, 0).reshape(C, T, 2, B)           # c, t, par, b
    ut2 = np.ascontiguousarray(utc.transpose(2, 0, 1, 3)).reshape(P, T, B)
    ut16 = ut2.astype(np.float16)
    dut16 = (ut2 - ut16.astype(np.float32)).astype(f8)
    ident32 = np.eye(P, dtype=np.float32)
    dup32 = np.tile(np.eye(B, dtype=np.float32), (1, 2))       # [64, 128]
    pull32 = np.concatenate(
        [np.zeros((B, B), np.float32), np.eye(B, dtype=np.float32)], axis=0
    )                                                          # [128, 64]
    in_maps = []
    Wf = np.asarray(W, dtype=np.float32)
    for k in range(NCORES):
        wk = Wf[:, k * NL:(k + 1) * NL]                        # [U, NL, C, D]
        wkt = wk.transpose(0, 2, 3, 1).reshape(T, 2, C, D * NL)  # t, par, c, dn
        wt2 = np.ascontiguousarray(wkt.transpose(1, 2, 0, 3)).reshape(P, T, DN)
        wt16 = wt2.astype(np.float16)
        dwt16 = (wt2 - wt16.astype(np.float32)).astype(f8)
        in_maps.append({"ut": ut16, "dut": dut16, "wt": wt16, "dwt": dwt16,
                        "ident32": ident32,
                        "dup32": dup32, "pull32": pull32})
    return in_maps


def kernel(u_vecs: np.ndarray, W: np.ndarray) -> np.ndarray:
    if "nc" not in _COMPILED:
        _COMPILED["nc"] = _build_program()
    nc = _COMPILED["nc"]
    in_maps = _prep_inputs(np.asarray(u_vecs), np.asarray(W))
    res = run_bass_kernel_spmd(nc, in_maps, list(range(NCORES)))
    outs = [np.asarray(res.results[k]["out"]) for k in range(NCORES)]
    return np.concatenate(outs, axis=-1).astype(np.float32)  # (B, D, N)


# revision 63
# speedup vs baseline: 1.8353x; 1.8353x over previous
"""Trainium2 Bass kernel for CapsNet dynamic routing (nn_Capsule_34342558498916).

Full inputs:  u_vecs (64, 64, 1024) f32, W (1024, 32, 64, 16) f32
Full output:  (64, 16, 32) f32  == transpose(v, (0, 2, 1)) of v (B, N, D)

Sharding: capsule dim N=32 split across 8 cores (NL=4 each); routing's
softmax axis (u) is core-local so no collectives.

Design notes (final):
  - The routing loop amplifies logit errors ~40x through exp (fp16
    anywhere in the u_hat path measures 2-5e-2 final error, over the 2e-2
    gate), so inputs are shipped as PAIRS: fp16 main values plus fp8e5m2
    RESIDUALS (the residuals are 2^-11-scale, so fp8 fidelity on them
    keeps the effective input error ~4e-5).  Phase-1 runs 3 matmul
    streams per output accumulating exactly in PSUM fp32: x16*w16 (fp16)
    + dx8*w8 + x8*dw8 (fp8, with the fp8 copies of x/w converted on-chip
    by ACT/GpSimd under the DMA window).  DMA is 25.2 MB/core, 0.75x of
    fp32 bytes, at fp16/fp8 matmul speed (1 cyc/row).
  - u_hat, e and pr2 are stored as float32r (fp32 range, ~11-bit
    mantissa; measured: final error matches an 11-bit-mantissa model at
    1.3e-2).  FP32R matmuls stream 1 cyc/row for >=256-row moving APs
    (fp32 proper would be 4).
  - Logits are recomputed per iteration from the accumulated v-sum
    (b_k = u_hat . (v1+..+v_{k-1}) by linearity) directly on the PE:
    stationary = diag(v[:, d, n]) multiplies each partition's resident
    u_hat stream inside the array, so the logits path materializes NO
    product tensor at all (the old mul1 pass, ~34us/iter of DVE, is gone).
    One PSUM tile per capsule n lets bmax/mneg/exp of capsule n pipeline
    under the folds of capsule n+1.
  - Per-parity softmax with deferred renormalization: each partition
    subtracts its own rowmax before exp (no cross-partition exchange on
    the critical path); the parity merge applies alpha = exp(m_par - M)
    to tiny [64, .] tensors afterwards.  All cross-partition moves run as
    small PE matmuls (dup/pull matrices), never SBUF-to-SBUF DMA.
  - exp runs on ACT with fused per-partition bias and fused Z accumulation
    (accum_out), reading logits straight from PSUM; sqrt in squash is
    exp(0.5*ln(x)) so ACT stays on one function table (the table list
    handed to the placement pass is filtered so Exp/Ln/Copy all resolve
    to natural_log_exp_and_others; one load at startup, no reloads).
  - s~ fold: pr2 = e * u_hat products split DVE/GpSimd per 32-t chunk,
    folded by an fp32r identity matmul into an 8-slot PSUM accumulator.
"""

import sys

import numpy as np

for _p in ("/opt/trn_rl_repo", "/opt/pypackages"):
    if _p not in sys.path:
        sys.path.append(_p)

import concourse.bass as bass
from concourse import bacc
import concourse.mybir as mybir
from concourse import tile
from concourse.bass_utils import run_bass_kernel_spmd

# Problem dims (hardcoded per harness contract)
B, C, U, N, D = 64, 64, 1024, 32, 16
NCORES = 8
NL = N // NCORES          # 4 capsules per core
T = U // 2                # 512 u-pairs
DN = D * NL               # 64 = phase-1 matmul free dim (d, n4)
P = 128
EPS = 1e-8
ROUTINGS = 3
CHT = 32                  # mul2 chunk (t per chunk)
NCH = T // CHT            # 16 chunks
DVT = 21                  # t per chunk on DVE (rest on GpSimd)

dt = mybir.dt
AF = mybir.ActivationFunctionType
ALU = mybir.AluOpType

_COMPILED = {}
F32R = dt.float32r


def _pin_act_tables():
    """Filter the ACT-table decision copy so Exp/Ln/Copy resolve to the one
    table containing all three.  Returns a restore function."""
    orig = bacc.get_activation_tables
    pin = {"Exp", "Ln", "Copy"}

    def patched(arch):
        tables = orig(arch)
        out = {}
        for name, fns in tables.items():
            if name == "natural_log_exp_and_others":
                out[name] = fns
            else:
                out[name] = {f for f in fns if str(f).split(".")[-1] not in pin}
        return out

    bacc.get_activation_tables = patched
    return lambda: setattr(bacc, "get_activation_tables", orig)


def _squash_core(nc, sm, s_red_ap, zi_or_scale, s_sb):
    """s_red_ap: un-normalized s~ [B, D, NL]; s = s~ * zi or * scalar;
    returns v_sb = squash(s).  sqrt(x) = exp(0.5*ln(x))."""
    if isinstance(zi_or_scale, float):
        nc.vector.tensor_scalar_mul(s_sb[:], s_red_ap, zi_or_scale)
    else:
        zi_bc = zi_or_scale[:].unsqueeze(1).broadcast_to([B, D, NL])
        nc.vector.tensor_mul(s_sb[:], s_red_ap, zi_bc)
    ssq = sm.tile([B, D, NL], dt.float32, tag="ssq")
    nc.vector.tensor_mul(ssq[:], s_sb[:], s_sb[:])
    s2 = sm.tile([B, NL], dt.float32, tag="s2")
    nc.vector.tensor_reduce(
        s2[:], ssq[:].transpose([0, 2, 1]), axis=mybir.AxisListType.X, op=ALU.add
    )
    s2e = sm.tile([B, NL], dt.float32, tag="s2e")
    nc.vector.tensor_scalar_add(s2e[:], s2[:], EPS)
    lns = sm.tile([B, NL], dt.float32, tag="lns")
    nc.scalar.activation(lns[:], s2e[:], AF.Ln)
    rt = sm.tile([B, NL], dt.float32, tag="rt")
    nc.scalar.activation(rt[:], lns[:], AF.Exp, scale=0.5)
    den = sm.tile([B, NL], dt.float32, tag="den")
    nc.vector.tensor_scalar_add(den[:], s2e[:], 1.0)
    deni = sm.tile([B, NL], dt.float32, tag="deni")
    nc.vector.reciprocal(deni[:], den[:])
    f = sm.tile([B, NL], dt.float32, tag="f")
    nc.vector.tensor_mul(f[:], rt[:], deni[:])
    v_sb = sm.tile([B, D, NL], dt.float32, tag="v_sb")
    nc.vector.tensor_mul(v_sb[:], s_sb[:], f[:].unsqueeze(1).broadcast_to([B, D, NL]))
    return v_sb


def _build_program():
    restore = _pin_act_tables()
    try:
        return _build_program_inner()
    finally:
        restore()


def _build_program_inner():
    nc = bacc.Bacc()

    ut = nc.dram_tensor("ut", [P, T, B], dt.float16, kind="ExternalInput")
    dut = nc.dram_tensor("dut", [P, T, B], dt.float8e5, kind="ExternalInput")
    wt = nc.dram_tensor("wt", [P, T, DN], dt.float16, kind="ExternalInput")
    dwt = nc.dram_tensor("dwt", [P, T, DN], dt.float8e5, kind="ExternalInput")
    ident32_d = nc.dram_tensor("ident32", [P, P], dt.float32, kind="ExternalInput")
    dup32_d = nc.dram_tensor("dup32", [B, P], dt.float32, kind="ExternalInput")
    pull32_d = nc.dram_tensor("pull32", [P, B], dt.float32, kind="ExternalInput")
    out_d = nc.dram_tensor("out", [B, D, NL], dt.float32, kind="ExternalOutput")

    with tile.TileContext(nc) as tc:
        with (
            tc.tile_pool(name="big", bufs=1) as big,
            tc.tile_pool(name="sm", bufs=1) as sm,
        ):
            u_hat = big.tile([P, T, D, NL], F32R, tag="u_hat")
            e_sb = big.tile([P, T, NL], F32R, tag="e_sb")
            identR = sm.tile([P, P], F32R, tag="identR")
            ident32 = sm.tile([P, P], dt.float32, tag="ident32")
            dup32 = sm.tile([B, P], dt.float32, tag="dup32")
            pull32 = sm.tile([P, B], dt.float32, tag="pull32")
            mnegs = []
            for n in range(NL):
                mneg_n = sm.tile([P, 1], dt.float32, tag=f"mneg{n}")
                mnegs.append(mneg_n)
            bmax = sm.tile([P, NL], dt.float32, tag="bmax")
            z_p = sm.tile([P, NL], dt.float32, tag="z_p")
            z_f = sm.tile([B, NL], dt.float32, tag="z_f")
            zi = sm.tile([B, NL], dt.float32, tag="zi")
            s_sb = sm.tile([B, D, NL], dt.float32, tag="s_sb")
            vsum = sm.tile([B, D, NL], dt.float32, tag="vsum")
            s1q = sm.tile([P, 16, DN], dt.float32, tag="s1q")

            # ---------------- Phase 1: u_hat = einsum over c ----------------
            # 3 fp16 streams accumulate per PSUM slot: x16*w16 + dx*w16 +
            # x16*dw (the dropped dx*dw term is ~2^-22 relative).
            TCH = 16   # t per DMA chunk
            with (
                tc.tile_pool(name="wts", bufs=6) as wts,
                tc.tile_pool(name="pp", bufs=3, space="PSUM") as pp,
            ):
                for ch in range(T // TCH):
                    sl = slice(ch * TCH, (ch + 1) * TCH)
                    wt_ch = wts.tile([P, TCH, DN], dt.float16, tag="wt_ch")
                    nc.sync.dma_start(wt_ch[:], wt[:, sl, :])
                    ut_ch = wts.tile([P, TCH, B], dt.float16, tag="ut_ch")
                    nc.sync.dma_start(ut_ch[:], ut[:, sl, :])
                    dwt_ch = wts.tile([P, TCH, DN], dt.float8e5, tag="dwt_ch")
                    nc.sync.dma_start(dwt_ch[:], dwt[:, sl, :])
                    dut_ch = wts.tile([P, TCH, B], dt.float8e5, tag="dut_ch")
                    nc.sync.dma_start(dut_ch[:], dut[:, sl, :])
                    # fp8 copies of u/w for the correction streams (the
                    # corrections are 2^-11-scale, so 12.5%-accurate fp8
                    # operands keep the effective input error ~4e-5)
                    ut8_ch = wts.tile([P, TCH, B], dt.float8e5, tag="ut8_ch")
                    nc.scalar.copy(ut8_ch[:], ut_ch[:])
                    wt8_ch = wts.tile([P, TCH, DN], dt.float8e5, tag="wt8_ch")
                    nc.gpsimd.tensor_copy(wt8_ch[:], wt_ch[:])
                    if ch == 0:
                        # constants ride behind the first input chunk so the
                        # PE's first matmuls are never queued behind them
                        nc.sync.dma_start(ident32[:], ident32_d[:])
                        nc.sync.dma_start(dup32[:], dup32_d[:])
                        nc.sync.dma_start(pull32[:], pull32_d[:])
                        # fp32r identity for the s~ fold (1.0/0.0 are exact
                        # in fp32r; the DVE copy provides the required
                        # fp32r-rounded producer)
                        nc.vector.tensor_copy(identR[:], ident32[:])
                    for g in range(TCH // 8):
                        ps = pp.tile([P, 8, DN], dt.float32, tag="pp")
                        for j in range(8):
                            tl = g * 8 + j
                            for half in (slice(0, 64), slice(64, 128)):
                                nc.tensor.matmul(
                                    ps[half, j, :], ut_ch[half, tl, :],
                                    wt_ch[half, tl, :], start=True, stop=False,
                                )
                                nc.tensor.matmul(
                                    ps[half, j, :], dut_ch[half, tl, :],
                                    wt8_ch[half, tl, :], start=False, stop=False,
                                )
                                nc.tensor.matmul(
                                    ps[half, j, :], ut8_ch[half, tl, :],
                                    dwt_ch[half, tl, :], start=False, stop=True,
                                )
                        t0 = ch * TCH + g * 8
                        dst = u_hat[:, t0:t0 + 8, :, :]
                        src = ps[:].rearrange("p e (d n) -> p e d n", d=D)
                        if (ch * 4 + g) % 2 == 0:
                            nc.scalar.copy(dst, src)
                        else:
                            nc.vector.tensor_copy(dst, src)
                    if ch % 2 == 1:
                        # s1 partial for this 32-t sixteenth (DVE reduce)
                        q = ch // 2
                        nc.vector.tensor_reduce(
                            s1q[:, q, :],
                            u_hat[:, q * 32:(q + 1) * 32, :, :]
                                .bitcast(dt.float32)
                                .rearrange("p t d n -> p (d n) t"),
                            axis=mybir.AxisListType.X, op=ALU.add,
                        )

            # routing-phase SBUF pools (opened after the wts pool closes)
            ctxP = tc.tile_pool(name="prod", bufs=3)
            prodp = ctxP.__enter__()
            ctxB = tc.tile_pool(name="big2", bufs=1)
            big2 = ctxB.__enter__()
            diags = big2.tile([P, NL, D, P], F32R, tag="diags")

            # v parity broadcast via PE dup matmul: psV[m,f] = src[m%64,f];
            # the diag builds read psV straight from PSUM.
            ctxV = tc.tile_pool(name="psV", bufs=1, space="PSUM")
            psVp = ctxV.__enter__()
            psV_ref = [None]

            def _vbc_update(src_f32):
                psV = psVp.tile([P, DN], dt.float32, tag="psV")
                nc.tensor.matmul(
                    psV[:], dup32[:], src_f32.rearrange("b d n -> b (d n)"),
                    start=True, stop=True,
                )
                psV_ref[0] = psV

            # ---------------- Iteration 1: uniform c -> v1 ----------------
            s1h = sm.tile([P, DN], dt.float32, tag="s1h")
            nc.vector.tensor_reduce(
                s1h[:], s1q[:].transpose([0, 2, 1]),
                axis=mybir.AxisListType.X, op=ALU.add,
            )
            # fold parities on PE (pull upper 64 down, add)
            ctxD = tc.tile_pool(name="psD", bufs=1, space="PSUM")
            psDp = ctxD.__enter__()
            ps1 = psDp.tile([B, DN], dt.float32, tag="ps1")
            nc.tensor.matmul(ps1[:], pull32[:], s1h[:], start=True, stop=True)
            s1f = sm.tile([B, D, NL], dt.float32, tag="s1f")
            nc.vector.tensor_add(
                s1f[:].rearrange("b d n -> b (d n)"), s1h[0:64, :], ps1[:]
            )
            v_sb = _squash_core(nc, sm, s1f[:], 1.0 / U, s_sb)
            nc.vector.tensor_copy(vsum[:], v_sb[:])
            _vbc_update(vsum[:])

            # ---------------- Iterations 2..3 ----------------
            ctx2 = tc.tile_pool(name="psL", bufs=1, space="PSUM")
            psL = ctx2.__enter__()
            ctx3 = tc.tile_pool(name="psS", bufs=1, space="PSUM")
            psS = ctx3.__enter__()
            for it in range(1, ROUTINGS):
                # build diag stationaries: diags[p, n, d, m] = v[p,d,n]*I[p,m]
                # logits[p,t,n] = sum_d v[p,d,n]*u_hat[p,t,d,n] on the PE.
                # One PSUM tile per n so bmax/mneg/exp of capsule n pipeline
                # under the folds of capsule n+1 (no whole-tile WAR).  The
                # diag build of n+1 and the bmax of n-1 interleave on the DVE
                # so neither blocks the softmax tail.
                psV = psV_ref[0]
                vv = psV[:].rearrange("p (d n) -> p d n", d=D)
                lg = []
                for n in range(NL):
                    lg_n = psL.tile([P, T], dt.float32, tag=f"lg{n}")
                    lg.append(lg_n)

                def _diag_build(n):
                    # two d-halves: the fold of capsule n only needs the
                    # d=0 rows first, so it can start after the first half
                    H = D // 2
                    for h in range(2):
                        nc.vector.tensor_mul(
                            diags[:, n, h * H:(h + 1) * H, :],
                            ident32[:].unsqueeze(1).broadcast_to([P, H, P]),
                            vv[:, h * H:(h + 1) * H, n].unsqueeze(2)
                                .broadcast_to([P, H, P]),
                        )

                def _softmax_tail(n):
                    nc.vector.tensor_reduce(
                        bmax[:, n:n + 1], lg[n][:].unsqueeze(1),
                        axis=mybir.AxisListType.X, op=ALU.max,
                    )
                    nc.vector.tensor_scalar_mul(
                        mnegs[n][:], bmax[:, n:n + 1], -1.0
                    )
                    # e = exp(b - parity rowmax) on ACT, per-parity Z fused
                    nc.scalar.activation(
                        e_sb[:, :, n], lg[n][:], AF.Exp,
                        bias=mnegs[n][:], scale=1.0,
                        accum_out=z_p[:, n:n + 1],
                    )

                _diag_build(0)
                _diag_build(1)
                for n in range(NL):
                    for d in range(D):
                        nc.tensor.matmul(
                            lg[n][:, :], diags[:, n, d, :],
                            u_hat[:, :, d, n],
                            start=(d == 0), stop=(d == D - 1),
                        )
                    if n + 2 < NL:
                        _diag_build(n + 2)
                    _softmax_tail(n)

                # s~ per parity: pr2 = e*u_hat (fp32r), identity fold on PE.
                # The parity merge (m/Z pulldowns + alpha/Z chain) is emitted
                # after the first two product chunks: it runs in the products'
                # shadow without delaying the first fold2 matmuls.
                psD = psDp.tile([B, 2 * NL + DN], dt.float32, tag="psD")
                s_ps = psS.tile([P, 8, D, NL], dt.float32, tag="s_ps")
                for c4 in range(NCH):
                    if c4 == 2:
                        nc.tensor.matmul(psD[:, 0:NL], pull32[:], bmax[:],
                                         start=True, stop=True)
                        nc.tensor.matmul(psD[:, NL:2 * NL], pull32[:], z_p[:],
                                         start=True, stop=True)
                        m_hi = psD[:, 0:NL]
                        z_hi = psD[:, NL:2 * NL]
                        # alpha_par = exp(m_par - M),  M = max(m_lo, m_hi)
                        Mfull = sm.tile([B, NL], dt.float32, tag="Mfull")
                        nc.vector.tensor_tensor(
                            Mfull[:], bmax[0:64, :], m_hi, op=ALU.max)
                        dlo = sm.tile([B, NL], dt.float32, tag="dlo")
                        nc.vector.tensor_sub(dlo[:], bmax[0:64, :], Mfull[:])
                        dhi = sm.tile([B, NL], dt.float32, tag="dhi")
                        nc.vector.tensor_sub(dhi[:], m_hi, Mfull[:])
                        alo = sm.tile([B, NL], dt.float32, tag="alo")
                        nc.scalar.activation(alo[:], dlo[:], AF.Exp)
                        ahi = sm.tile([B, NL], dt.float32, tag="ahi")
                        nc.scalar.activation(ahi[:], dhi[:], AF.Exp)
                        # Z = alo*z_lo + ahi*z_hi ; zi = 1/Z
                        zt0 = sm.tile([B, NL], dt.float32, tag="zt0")
                        nc.vector.tensor_mul(zt0[:], z_p[0:64, :], alo[:])
                        zt1 = sm.tile([B, NL], dt.float32, tag="zt1")
                        nc.vector.tensor_mul(zt1[:], z_hi, ahi[:])
                        nc.vector.tensor_add(z_f[:], zt0[:], zt1[:])
                        nc.vector.reciprocal(zi[:], z_f[:])
                    tb = c4 * CHT
                    pr2 = prodp.tile([P, CHT, D, NL], F32R, tag="pr2")
                    nc.vector.tensor_mul(
                        pr2[:, 0:DVT, :, :],
                        u_hat[:, tb:tb + DVT, :, :].bitcast(dt.float32),
                        e_sb[:, tb:tb + DVT, :].bitcast(dt.float32).unsqueeze(2)
                            .broadcast_to([P, DVT, D, NL]),
                    )
                    nc.gpsimd.tensor_mul(
                        pr2[:, DVT:CHT, :, :],
                        u_hat[:, tb + DVT:tb + CHT, :, :].bitcast(dt.float32),
                        e_sb[:, tb + DVT:tb + CHT, :].bitcast(dt.float32)
                            .unsqueeze(2)
                            .broadcast_to([P, CHT - DVT, D, NL]),
                    )
                    for g in range(CHT // 8):
                        nc.tensor.matmul(
                            s_ps[:], identR[:], pr2[:, g * 8:(g + 1) * 8, :, :],
                            start=(c4 == 0 and g == 0),
                            stop=(c4 == NCH - 1 and g == CHT // 8 - 1),
                        )
                # fold slots per parity: s_pp [128, D, NL]
                s_pp = sm.tile([P, D, NL], dt.float32, tag="s_pp")
                nc.vector.tensor_reduce(
                    s_pp[:], s_ps[:].transpose([0, 2, 3, 1]),
                    axis=mybir.AxisListType.X, op=ALU.add,
                )
                # pull upper-parity s down to rows 0:64 on the PE
                nc.tensor.matmul(
                    psD[:, 2 * NL:], pull32[:],
                    s_pp[:].rearrange("p d n -> p (d n)"),
                    start=True, stop=True,
                )
                s_hi = psD[:, 2 * NL:].rearrange("b (d n) -> b d n", d=D)
                # s_red = alo*s_lo + ahi*s_hi
                st0 = sm.tile([B, D, NL], dt.float32, tag="st0")
                nc.vector.tensor_mul(
                    st0[:], s_pp[0:64, :, :],
                    alo[:].unsqueeze(1).broadcast_to([B, D, NL]),
                )
                s_red = sm.tile([B, D, NL], dt.float32, tag="s_red")
                nc.vector.tensor_mul(
                    s_red[:], s_hi,
                    ahi[:].unsqueeze(1).broadcast_to([B, D, NL]),
                )
                nc.vector.tensor_add(s_red[:], s_red[:], st0[:])
                v_sb = _squash_core(nc, sm, s_red[:], zi, s_sb)
                if it < ROUTINGS - 1:
                    nc.vector.tensor_add(vsum[:], vsum[:], v_sb[:])
                    _vbc_update(vsum[:])

            nc.sync.dma_start(out_d[:], v_sb[:])
            ctx3.__exit__(None, None, None)
            ctx2.__exit__(None, None, None)
            ctxD.__exit__(None, None, None)
            ctxV.__exit__(None, None, None)
            ctxB.__exit__(None, None, None)
            ctxP.__exit__(None, None, None)

    nc.finalize()
    return nc


def _prep_inputs(u_vecs, W):
    """Host-side shard + relayout.  fp16 pairs (value + residual)."""
    import ml_dtypes
    f8 = ml_dtypes.float8_e5m2
    u32 = np.asarray(u_vecs, dtype=np.float32)
    utc = u32.transpose(1, 2, 0).reshape(C, T, 2, B)           # c, t, par, b
    ut2 = np.ascontiguousarray(utc.transpose(2, 0, 1, 3)).reshape(P, T, B)
    ut16 = ut2.astype(np.float16)
    dut16 = (ut2 - ut16.astype(np.float32)).astype(f8)
    ident32 = np.eye(P, dtype=np.float32)
    dup32 = np.tile(np.eye(B, dtype=np.float32), (1, 2))       # [64, 128]
    pull32 = np.concatenate(
        [np.zeros((B, B), np.float32), np.eye(B, dtype=np.float32)], axis=0
    )                                                          # [128, 64]
    in_maps = []
    Wf = np.asarray(W, dtype=np.float32)
    for k in range(NCORES):
        wk = Wf[:, k * NL:(k + 1) * NL]                        # [U, NL, C, D]
        wkt = wk.transpose(0, 2, 3, 1).reshape(T, 2, C, D * NL)  # t, par, c, dn
        wt2 = np.ascontiguousarray(wkt.transpose(1, 2, 0, 3)).reshape(P, T, DN)
        wt16 = wt2.astype(np.float16)
        dwt16 = (wt2 - wt16.astype(np.float32)).astype(f8)
        in_maps.append({"ut": ut16, "dut": dut16, "wt": wt16, "dwt": dwt16,
                        "ident32": ident32,
                        "dup32": dup32, "pull32": pull32})
    return in_maps


def kernel(u_vecs: np.ndarray, W: np.ndarray) -> np.ndarray:
    if "nc" not in _COMPILED:
        _COMPILED["nc"] = _build_program()
    nc = _COMPILED["nc"]
    in_maps = _prep_inputs(np.asarray(u_vecs), np.asarray(W))
    res = run_bass_kernel_spmd(nc, in_maps, list(range(NCORES)))
    outs = [np.asarray(res.results[k]["out"]) for k in range(NCORES)]
    return np.concatenate(outs, axis=-1).astype(np.float32)  # (B, D, N)
